# revision 6
# baseline (speedup 1.0000x reference)
"""Distributed Trainium2 kernel for nn_Attention_29832842838194.

LayerNorm (stats over the *sequence* axis) -> QKV projection -> 8-head
attention (N=2048, d_head=64) -> output projection, on 8 NeuronCores.

Sharding (v2 — token-parallel QKV, head-parallel attention):
  - tokens (B*N = 4096) split 8 ways; core c owns tokens [c*512, (c+1)*512)
    (all of one batch), computes LN partial stats and the full QKV
    projection for its tokens (M=128 full-PE-width matmuls).
  - LN stats: one batch-group AllReduce (groups {0..3}, {4..7}) of the
    16KB packed (sum, sumsq) vector — first collective, triggered ~10us
    in so the inter-core launch skew is absorbed while weights stream.
  - qkv reshard: three 512KB AllToAlls (k, then q, then v), each
    triggered as soon as its 4 M-chunks of the projection finish, so
    transfer overlaps the remaining matmuls. After the A2A core c holds
    q,k,v for head c over all 4096 tokens.
  - attention: head-parallel; sim matmuls (K=64) packed 2-at-a-time into
    the 128x128 PE via row tiling (kT/qT duplicated to partitions
    64..127); exp on ScalarE from fp32 PSUM with 1/sqrt(d) folded in;
    PV accumulates [65, 512] (65th row = softmax denominator).
  - output reshard: one 266KB AllToAll per batch; batch 0's normalize +
    out-projection is emitted under batch 1's attention.

Numerics: all matmuls bf16 with fp32 PSUM accumulation (matches the
baseline's accuracy envelope).

The kernel is self-contained: shapes are hardcoded to the problem spec.
"""

import numpy as np

# -------- problem constants (hardcoded per spec) --------
B = 2
NSEQ = 2048  # sequence length per batch
DIM = 1024
HEADS = 8
DHEAD = 64
INNER = HEADS * DHEAD  # 512
EPS = 1e-5
NCORES = 8
P = 128

SCALE = DHEAD ** -0.5  # 0.125


def _cfg(nseq=NSEQ):
    """Derived dims. nseq can be shrunk for simulator tests."""
    T = B * nseq              # total tokens
    TLOC = T // NCORES        # tokens per core
    TB = nseq // NCORES       # tokens per core per batch (out-a2a shard)
    assert TLOC % P == 0 and nseq % 512 == 0
    return dict(
        nseq=nseq,
        T=T,
        TLOC=TLOC,
        TB=TB,
        XT=TLOC // P,         # x token-subtiles per core (4)
        DC=DIM // P,          # 8 dmodel chunks
        KC=INNER // P,        # 4 inner chunks
        MC=3 * INNER // P,    # 12 qkv output chunks
        IB=nseq // 512,       # 512-query i-blocks per batch (4)
        JC=nseq // P,         # 128-key j-chunks per batch (16)
    )


def build_body(tc, outs, ins, cfg, dbg=False):
    """Emit the per-core program. outs/ins are dicts of DRAM APs."""
    import concourse.mybir as mybir
    from concourse.masks import make_identity
    from concourse.tile import add_dep_helper

    dt = mybir.dt
    AF = mybir.ActivationFunctionType
    ALU = mybir.AluOpType
    nc = tc.nc

    T, TLOC, TB, XT = cfg["T"], cfg["TLOC"], cfg["TB"], cfg["XT"]
    DC, KC, MC, IB, JC = cfg["DC"], cfg["KC"], cfg["MC"], cfg["IB"], cfg["JC"]
    nseq = cfg["nseq"]
    NTOK = float(nseq)  # tokens per batch (LN normalizer)
    RG = [list(range(NCORES))]
    # batch groups: cores 0..3 hold batch 0's tokens, 4..7 batch 1's
    half = NCORES // 2
    RGB = [list(range(half)), list(range(half, NCORES))]
    DE = DHEAD + 1

    x = ins["x"]      # [TLOC, DIM] f32 (this core's token slice)
    g = ins["g"]      # [DIM] f32
    w = ins["w"]      # [DIM, 3*INNER] f32: columns [k | q | v], head-major
    wo = ins["wo"]    # [INNER, DIM] f32 (replicated)
    out = outs["out"]  # [2*TB, DIM] f32 (rows: b*TB + t)

    with (
        tc.tile_pool(name="persist", bufs=1) as pp,
        tc.tile_pool(name="work", bufs=3) as pool,
        tc.tile_pool(name="work2", bufs=2) as pool2,
        tc.tile_pool(name="pref", bufs=16) as pref,
        tc.tile_pool(name="psum", bufs=2, space="PSUM") as psum,
        tc.tile_pool(name="dram", bufs=1, space="DRAM") as dram,
    ):
        # -------- dummy rendezvous collective: the FIRST cc op in a program
        # pays a ~44us one-time global-comm setup/rendezvous. Fire a tiny
        # AllReduce immediately so that cost runs concurrent with the whole
        # prologue instead of stalling the LN-stats AllReduce.
        dummy_in = dram.tile([1, 8], dt.float32, tag="dmyi")
        dummy_out = dram.tile([1, 8], dt.float32, tag="dmyo")
        nc.gpsimd.collective_compute(
            "AllReduce", ALU.add, replica_groups=RG,
            ins=[dummy_in.opt()], outs=[dummy_out.opt()],
        )

        # -------- constants
        ident = pp.tile([P, P], dt.bfloat16)
        make_identity(nc, ident)
        g_sb = pp.tile([P, DC], dt.float32)
        nc.sync.dma_start(g_sb[:], g.rearrange("(c p) -> p c", p=P))

        # -------- phase 0: x load -> cast -> transpose; LN partial stats
        # pipelined per d-chunk straight out of PSUM so the group
        # AllReduce triggers ASAP (the first collective absorbs skew)
        x_bf = pp.tile([P, XT, DIM], dt.bfloat16)
        for t in range(XT):
            x_f = pool.tile([P, DIM], dt.float32, tag="xload")
            nc.sync.dma_start(x_f[:], x[t * P:(t + 1) * P, :])
            nc.vector.tensor_copy(x_bf[:, t, :], x_f[:])
        # xT[p, dc, t] = x[t, dc*128+p] (bf16)
        xT = pp.tile([P, DC, TLOC], dt.bfloat16)
        ar_sb = pp.tile([P, 2 * DC], dt.float32)
        for dc in range(DC):
            ps = psum.tile([P, TLOC], dt.bfloat16, tag="tr")
            for t in range(XT):
                nc.tensor.transpose(
                    ps[:, t * P:(t + 1) * P],
                    x_bf[:, t, dc * P:(dc + 1) * P],
                    ident[:],
                )
            nc.vector.tensor_reduce(
                ar_sb[:, dc:dc + 1], ps[:], axis=mybir.AxisListType.X,
                op=ALU.add,
            )
            trash = pool2.tile([P, TLOC], dt.float32, tag="trash")
            nc.scalar.activation(
                trash[:], ps[:], AF.Square,
                accum_out=ar_sb[:, DC + dc:DC + dc + 1],
            )
            nc.vector.tensor_copy(xT[:, dc, :], ps[:])
        ar_in = dram.tile([P, 2 * DC], dt.float32)
        ar_dma = nc.sync.dma_start(ar_in[:], ar_sb[:])
        ar_out = dram.tile([P, 2 * DC], dt.float32, tag="arout")
        nc.gpsimd.collective_compute(
            "AllReduce", ALU.add, replica_groups=RGB,
            ins=[ar_in.opt()], outs=[ar_out.opt()],
        )

        # -------- weights (stream + cast while the AllReduce flies).
        # Gate the 8.4MB of weight DMAs behind the tiny AR-input DMA so
        # they don't steal HBM bandwidth from the stats critical path.
        w_bf = pp.tile([P, DC, 3 * INNER], dt.bfloat16)
        for kc in range(DC):
            wl = pool.tile([P, 3 * INNER], dt.float32, tag="wload")
            wdma = nc.sync.dma_start(wl[:], w[kc * P:(kc + 1) * P, :])
            add_dep_helper(wdma.ins, ar_dma.ins,
                           reason="defer weight loads behind stats AR input")
            nc.vector.tensor_copy(w_bf[:, kc, :], wl[:])
        wo_bf = pp.tile([P, KC, DIM], dt.bfloat16)
        for kc in range(KC):
            wol = pool.tile([P, DIM], dt.float32, tag="wload")
            wdma = nc.sync.dma_start(wol[:], wo[kc * P:(kc + 1) * P, :])
            add_dep_helper(wdma.ins, ar_dma.ins,
                           reason="defer weight loads behind stats AR input")
            nc.vector.tensor_copy(wo_bf[:, kc, :], wol[:])

        # head-broadcast selector for the rownorm: sel[h, kc, m] = 1 iff
        # h == 2*kc + (m >= DHEAD)
        sel_np = np.zeros((NCORES, KC, P), np.float32)
        for kc in range(KC):
            sel_np[2 * kc, kc, 0:DHEAD] = 1.0
            sel_np[2 * kc + 1, kc, DHEAD:P] = 1.0
        sel_dram = nc.inline_tensor(sel_np, name="selmat")
        sel = pp.tile([NCORES, KC, P], dt.float32)
        nc.sync.dma_start(sel[:], sel_dram.ap())

        # -------- LN coefficients (this core's batch only)
        stats = pp.tile([P, 2 * DC], dt.float32)
        nc.sync.dma_start(stats[:], ar_out[:])
        mean = pp.tile([P, DC], dt.float32)
        nc.vector.tensor_scalar_mul(mean[:], stats[:, 0:DC], 1.0 / NTOK)
        e2 = pp.tile([P, DC], dt.float32)
        nc.vector.tensor_scalar_mul(e2[:], stats[:, DC:2 * DC], 1.0 / NTOK)
        msq = pp.tile([P, DC], dt.float32)
        nc.vector.tensor_tensor(msq[:], mean[:], mean[:], ALU.mult)
        vareps = pp.tile([P, DC], dt.float32)
        nc.vector.tensor_tensor(vareps[:], e2[:], msq[:], ALU.subtract)
        nc.vector.tensor_scalar_add(vareps[:], vareps[:], EPS)
        rvar = pp.tile([P, DC], dt.float32)
        nc.vector.reciprocal(rvar[:], vareps[:])
        rstd = pp.tile([P, DC], dt.float32)
        nc.scalar.activation(rstd[:], rvar[:], AF.Sqrt)
        A2 = pp.tile([P, DC], dt.float32)
        nc.vector.tensor_tensor(A2[:], rstd[:], g_sb[:], ALU.mult)
        C2 = pp.tile([P, DC], dt.float32)
        nc.vector.tensor_tensor(C2[:], mean[:], A2[:], ALU.mult)
        nc.vector.tensor_scalar_mul(C2[:], C2[:], -1.0)
        # preload the exp table while the PE chews on QKV
        junk = pp.tile([1, DC], dt.float32)
        nc.scalar.activation(junk[:], A2[0:1, :], AF.Exp)
        # warm the PE (HAM throttle) during the LN-coefficient window so
        # QKV runs at 2.4GHz: ~3.4us of junk matmuls gated on the AR
        stats_bf = pp.tile([P, 2 * DC], dt.bfloat16)
        nc.vector.tensor_copy(stats_bf[:], stats[:])
        for wu in range(8):
            jp = psum.tile([2 * DC, TLOC], dt.float32, tag="tr")
            nc.tensor.matmul(jp[:], stats_bf[:], xT[:, wu % DC, :],
                             start=True, stop=True)
        if dbg:
            nc.sync.dma_start(outs["dbg_stats"], stats[:])
            nc.sync.dma_start(outs["dbg_A2"], A2[:])
            nc.sync.dma_start(outs["dbg_C2"], C2[:])

        # normalize xT in place
        for dc in range(DC):
            nc.vector.tensor_scalar(
                xT[:, dc, :], xT[:, dc, :],
                A2[:, dc:dc + 1], C2[:, dc:dc + 1],
                ALU.mult, ALU.add,
            )

        # -------- QKV projection (12 M-chunks; w columns are [k | q | v]).
        # The k AllToAll fires as soon as its 4 M-chunks finish, q after the
        # next 4, v last — each transfer overlaps the remaining projection
        # matmuls, so attention can start right after the q A2A lands.
        k_in = dram.tile([NCORES, DHEAD, TLOC], dt.bfloat16, tag="ki")
        q_in = dram.tile([NCORES, DHEAD, TLOC], dt.bfloat16, tag="qi")
        v_in = dram.tile([NCORES, TLOC, DHEAD], dt.bfloat16, tag="vi")
        k_out = dram.tile([NCORES, DHEAD, TLOC], dt.bfloat16, tag="ko")
        q_out = dram.tile([NCORES, DHEAD, TLOC], dt.bfloat16, tag="qo")
        v_out = dram.tile([NCORES, TLOC, DHEAD], dt.bfloat16, tag="vo")
        for mc in range(2 * KC):
            qp = psum.tile([P, TLOC], dt.float32, tag="acc")
            for kc in range(DC):
                nc.tensor.matmul(
                    qp[:], w_bf[:, kc, mc * P:(mc + 1) * P], xT[:, kc, :],
                    start=(kc == 0), stop=(kc == DC - 1),
                )
            qsb = pool.tile([P, TLOC], dt.bfloat16, tag="qsb")
            nc.vector.tensor_copy(qsb[:], qp[:])
            grp, mg = divmod(mc, KC)
            dst = k_in if grp == 0 else q_in
            for hh in range(2):
                nc.sync.dma_start(dst[2 * mg + hh],
                                  qsb[hh * DHEAD:(hh + 1) * DHEAD, :])
            if mc == KC - 1:
                nc.gpsimd.collective_compute(
                    "AllToAll", ALU.bypass, replica_groups=RG,
                    ins=[k_in.opt()], outs=[k_out.opt()],
                )
        nc.gpsimd.collective_compute(
            "AllToAll", ALU.bypass, replica_groups=RG,
            ins=[q_in.opt()], outs=[q_out.opt()],
        )
        # v computed pre-transposed ([token, vcol]) on the sender, so the
        # receiver needs no PE transposes before PV can start
        for tc in range(XT):
            vp = psum.tile([P, INNER], dt.float32, tag="acc")
            for kc in range(DC):
                nc.tensor.matmul(
                    vp[:], xT[:, kc, tc * P:(tc + 1) * P],
                    w_bf[:, kc, 2 * INNER:3 * INNER],
                    start=(kc == 0), stop=(kc == DC - 1),
                )
            vsb = pool.tile([P, INNER], dt.bfloat16, tag="qsb")
            nc.vector.tensor_copy(vsb[:], vp[:])
            nc.sync.dma_start(
                v_in[:, tc * P:(tc + 1) * P, :].rearrange("h p d -> p h d"),
                vsb[:].rearrange("p (h d) -> p h d", h=NCORES),
            )
        nc.gpsimd.collective_compute(
            "AllToAll", ALU.bypass, replica_groups=RG,
            ins=[v_in.opt()], outs=[v_out.opt()],
        )

        # gather: kT2/qT2 [128, T] with rows 64..127 duplicating 0..63
        # (row-tiled sim runs two j-chunks concurrently); vT [65, T] with
        # the ones row for the softmax denominator.
        kT2 = pp.tile([P, T], dt.bfloat16)
        qT2 = pp.tile([P, T], dt.bfloat16)
        k_r = k_out.rearrange("r d t -> d r t")
        q_r = q_out.rearrange("r d t -> d r t")

        def _split(ap):
            return ap.rearrange("d (r t) -> d r t", r=NCORES)

        nc.sync.dma_start(_split(kT2[0:DHEAD, :]), k_r)
        nc.sync.dma_start(_split(kT2[DHEAD:P, :]), k_r)
        nc.sync.dma_start(_split(qT2[0:DHEAD, :]), q_r)
        nc.sync.dma_start(_split(qT2[DHEAD:P, :]), q_r)
        # vext[j, jc, d|1]: ones column preset, data straight off the A2A
        NJC = T // P  # 32 j-chunks over both batches
        vext = pp.tile([P, NJC, DE], dt.bfloat16)
        nc.gpsimd.memset(vext[:, :, DHEAD:DE], 1.0)
        nc.sync.dma_start(
            vext[:, :, 0:DHEAD],
            v_out.rearrange("r (jc p) d -> p (r jc) d", p=P),
        )
        if dbg:
            nc.sync.dma_start(outs["dbg_xn"], xT[:])
            nc.sync.dma_start(outs["dbg_kT"], kT2[:])
            nc.sync.dma_start(outs["dbg_qT"], qT2[:])
            nc.sync.dma_start(outs["dbg_vT"], vext[:])

        # -------- attention + output A2As, postprocess interleaved
        aoT = pp.tile([DE, T], dt.bfloat16)

        def attn_sims(b, ib, et_pool, et_tag):
            """Emit sims + exps only (for the window where v is in flight)."""
            i0 = b * nseq + ib * 512
            ets = []
            for jg in range(JC // 2):
                sp = psum.tile([P, 1024], dt.float32, tag="sim")
                for u in range(2):
                    j0 = b * nseq + (jg * 2 + u) * P
                    rsl = slice(u * DHEAD, u * DHEAD + DHEAD)
                    nc.tensor.matmul(
                        sp[:, u * 512:(u + 1) * 512],
                        kT2[rsl, j0:j0 + P], qT2[rsl, i0:i0 + 512],
                        start=True, stop=True,
                    )
                et = et_pool.tile([P, 1024], dt.bfloat16, tag=et_tag)
                nc.scalar.activation(et[:], sp[:], AF.Exp, scale=SCALE)
                ets.append(et)
            return ets

        def attn_pvs(b, ib, ets):
            i0 = b * nseq + ib * 512
            otp = psum.tile([DE, 512], dt.float32, tag="acc")
            for jg in range(JC // 2):
                for u in range(2):
                    jc = jg * 2 + u
                    nc.tensor.matmul(
                        otp[:], vext[:, b * JC + jc, :],
                        ets[jg][:, u * 512:(u + 1) * 512],
                        start=(jg == 0 and u == 0),
                        stop=(jg == JC // 2 - 1 and u == 1),
                    )
            nc.vector.tensor_copy(aoT[:, i0:i0 + 512], otp[:])

        def attn_block(b, ib):
            i0 = b * nseq + ib * 512
            otp = psum.tile([DE, 512], dt.float32, tag="acc")
            for jg in range(JC // 2):
                sp = psum.tile([P, 1024], dt.float32, tag="sim")
                for u in range(2):
                    j0 = b * nseq + (jg * 2 + u) * P
                    rsl = slice(u * DHEAD, u * DHEAD + DHEAD)
                    nc.tensor.matmul(
                        sp[:, u * 512:(u + 1) * 512],
                        kT2[rsl, j0:j0 + P], qT2[rsl, i0:i0 + 512],
                        start=True, stop=True,
                    )
                et = pool.tile([P, 1024], dt.bfloat16, tag="exp")
                nc.scalar.activation(et[:], sp[:], AF.Exp, scale=SCALE)
                for u in range(2):
                    jc = jg * 2 + u
                    nc.tensor.matmul(
                        otp[:], vext[:, b * JC + jc, :],
                        et[:, u * 512:(u + 1) * 512],
                        start=(jg == 0 and u == 0),
                        stop=(jg == JC // 2 - 1 and u == 1),
                    )
            nc.vector.tensor_copy(aoT[:, i0:i0 + 512], otp[:])

        def out_a2a(off, tb, tag):
            a2a_in = dram.tile([NCORES, DE, tb], dt.bfloat16,
                               tag=f"oa{tag}")
            nc.sync.dma_start(
                a2a_in.rearrange("s d t -> d s t"),
                aoT[:, off:off + NCORES * tb].rearrange(
                    "d (s t) -> d s t", s=NCORES),
            )
            a2a_out = dram.tile([NCORES, DE, tb], dt.bfloat16,
                                tag=f"ob{tag}")
            nc.gpsimd.collective_compute(
                "AllToAll", ALU.bypass, replica_groups=RG,
                ins=[a2a_in.opt()], outs=[a2a_out.opt()],
            )
            return a2a_out

        def pp_stages(row0, tb, a2a_out):
            """Postprocess split into stages so the PE work trickles into
            the exp-stream's slack instead of lumping into one stall."""
            st = {}

            def s_gather():
                ao_g = pool2.tile([P, KC, tb], dt.bfloat16, tag="aog")
                a2a_v = a2a_out.rearrange("(kc rr) d t -> rr d kc t", rr=2)
                for rr in range(2):
                    nc.scalar.dma_start(
                        ao_g[rr * DHEAD:(rr + 1) * DHEAD, :, :],
                        a2a_v[rr, 0:DHEAD],
                    )
                rn = pool2.tile([NCORES, tb], dt.bfloat16, tag="rn")
                nc.scalar.dma_start(rn[:], a2a_out[:, DHEAD, :])
                rc = pool2.tile([NCORES, tb], dt.float32, tag="rc")
                nc.vector.reciprocal(rc[:], rn[:])
                st["ao_g"], st["rc"] = ao_g, rc

            def s_norm():
                ao_g, rc = st["ao_g"], st["rc"]
                for kc in range(KC):
                    bcp = psum.tile([P, tb], dt.float32, tag="tr")
                    nc.tensor.matmul(bcp[:], sel[:, kc, :], rc[:],
                                     start=True, stop=True)
                    nc.vector.tensor_tensor(
                        ao_g[:, kc, :], ao_g[:, kc, :], bcp[:], ALU.mult
                    )

            def s_proj(t0):
                ao_g = st["ao_g"]
                mw = min(P, tb - t0)
                out_sb = pool2.tile([P, DIM], dt.float32, tag="osb")
                for nh2 in range(DIM // 512):
                    op = psum.tile([P, 512], dt.float32, tag="tr")
                    for kc in range(KC):
                        nc.tensor.matmul(
                            op[0:mw, :], ao_g[:, kc, t0:t0 + mw],
                            wo_bf[:, kc, nh2 * 512:(nh2 + 1) * 512],
                            start=(kc == 0), stop=(kc == KC - 1),
                        )
                    nc.vector.tensor_copy(
                        out_sb[0:mw, nh2 * 512:(nh2 + 1) * 512], op[0:mw, :]
                    )
                nc.scalar.dma_start(out[row0 + t0:row0 + t0 + mw, :],
                                    out_sb[0:mw, :])

            stages = [s_gather, s_norm]
            stages += [lambda t0=t0: s_proj(t0) for t0 in range(0, tb, P)]
            return stages

        TB1 = (TB * 3) // 4             # b1 first shard (192 tokens/core)
        TB2 = TB - TB1                  # b1 final shard kept small (64)
        # first two i-blocks: sims+exps run while the v A2A is in flight,
        # PVs follow once vext lands
        ets0 = attn_sims(0, 0, pref, "pref0")
        ets1 = attn_sims(0, 1, pref, "pref1")
        attn_pvs(0, 0, ets0)
        attn_pvs(0, 1, ets1)
        attn_block(0, 2)
        attn_block(0, 3)
        o0 = out_a2a(0, TB, "b0")
        pp0 = pp_stages(0, TB, o0)
        attn_block(1, 0)
        pp0[0]()                        # gather+rc (no PE work)
        attn_block(1, 1)
        pp0[1]()                        # rownorm broadcast
        o10 = out_a2a(nseq, TB1, "b1h0")
        pp10 = pp_stages(TB, TB1, o10)
        attn_block(1, 2)
        pp0[2]()                        # b0 out-projection, first half
        attn_block(1, 3)
        o11 = out_a2a(nseq + NCORES * TB1, TB2, "b1h1")
        pp11 = pp_stages(TB + TB1, TB2, o11)
        pp0[3]()                        # b0 out-projection, second half
        for s in pp10:                  # b1 first shard, under b1h1's A2A
            s()
        for s in pp11:
            s()
        if dbg:
            nc.sync.dma_start(outs["dbg_aoT"], aoT[:])


def build_graph(cfg, dbg=False):
    import concourse.mybir as mybir
    import concourse.tile as tile
    from concourse import bacc

    dt = mybir.dt
    nc = bacc.Bacc("TRN2", target_bir_lowering=False, debug=False,
                   num_devices=NCORES)
    TLOC, TB = cfg["TLOC"], cfg["TB"]
    T, DC = cfg["T"], cfg["DC"]
    ins = {
        "x": nc.dram_tensor("x", [TLOC, DIM], dt.float32, kind="ExternalInput").ap(),
        "g": nc.dram_tensor("g", [DIM], dt.float32, kind="ExternalInput").ap(),
        "w": nc.dram_tensor("w", [DIM, 3 * INNER], dt.float32, kind="ExternalInput").ap(),
        "wo": nc.dram_tensor("wo", [INNER, DIM], dt.float32, kind="ExternalInput").ap(),
    }
    outs = {
        "out": nc.dram_tensor("out", [B * TB, DIM], dt.float32,
                              kind="ExternalOutput").ap(),
    }
    if dbg:
        for name, shape, dt_ in (
            ("dbg_stats", [P, 2 * DC], dt.float32),
            ("dbg_A2", [P, DC], dt.float32),
            ("dbg_C2", [P, DC], dt.float32),
            ("dbg_xn", [P, DC, TLOC], dt.bfloat16),
            ("dbg_kT", [P, T], dt.bfloat16),
            ("dbg_qT", [P, T], dt.bfloat16),
            ("dbg_vT", [P, T // P, DHEAD + 1], dt.bfloat16),
            ("dbg_aoT", [DHEAD + 1, T], dt.bfloat16),
        ):
            outs[name] = nc.dram_tensor(name, shape, dt_,
                                        kind="ExternalOutput").ap()
    with tile.TileContext(nc) as tc:
        build_body(tc, outs, ins, cfg, dbg=dbg)
    nc.compile()
    return nc


def make_in_maps(x, g, wq, wkv, wo, cfg):
    """Shard full inputs into per-core input maps."""
    T, TLOC = cfg["T"], cfg["TLOC"]
    x2 = np.ascontiguousarray(np.asarray(x, np.float32).reshape(T, DIM))
    g_ = np.ascontiguousarray(np.asarray(g, np.float32))
    wq_ = np.asarray(wq, np.float32)
    wkv_ = np.asarray(wkv, np.float32)
    wo_ = np.ascontiguousarray(np.asarray(wo, np.float32))
    # columns [k | q | v], head-major inside each block
    w_cat = np.ascontiguousarray(
        np.concatenate([wkv_[:, :INNER], wq_, wkv_[:, INNER:]], axis=1)
    )
    in_maps = []
    for c in range(NCORES):
        in_maps.append({
            "x": np.ascontiguousarray(x2[c * TLOC:(c + 1) * TLOC]),
            "g": g_,
            "w": w_cat,
            "wo": wo_,
        })
    return in_maps


def assemble_out(core_outs, cfg):
    """Batch 0 resharded in TB-token shards, batch 1 in a 3/4 + 1/4 split."""
    T, TB = cfg["T"], cfg["TB"]
    nseq = cfg["nseq"]
    TB1 = (TB * 3) // 4
    TB2 = TB - TB1
    full = np.empty((T, DIM), np.float32)
    for c in range(NCORES):
        o = core_outs[c]
        full[c * TB:(c + 1) * TB] = o[0:TB]
        for src, dst, n in (
            (TB, nseq + c * TB1, TB1),
            (TB + TB1, nseq + NCORES * TB1 + c * TB2, TB2),
        ):
            full[dst:dst + n] = o[src:src + n]
    return full


_cache = {}


def _get_graph():
    if "nc" not in _cache:
        _cache["nc"] = build_graph(_cfg())
    return _cache["nc"]


def run_on_hw(in_maps, trace=False, **kw):
    from concourse.bass_utils import run_bass_kernel_spmd
    nc = _get_graph()
    return run_bass_kernel_spmd(
        nc, in_maps, core_ids=list(range(NCORES)), trace=trace, **kw
    )


def kernel(x, g, wq, wkv, wo):
    cfg = _cfg()
    in_maps = make_in_maps(x, g, wq, wkv, wo, cfg)
    res = run_on_hw(in_maps)
    core_outs = [np.asarray(res.results[c]["out"], np.float32)
                 for c in range(NCORES)]
    return assemble_out(core_outs, cfg).reshape(B, NSEQ, DIM)



# revision 19
# speedup vs baseline: 1.0283x; 1.0283x over previous
"""Distributed Trainium2 kernel for nn_Attention_29832842838194.

LayerNorm (stats over the *sequence* axis) -> QKV projection -> 8-head
attention (N=2048, d_head=64) -> output projection, on 8 NeuronCores.

Sharding (v3 — one batch + two heads per core, collective-free front):
  - core c owns batch c//4 and heads {2*(c%4), 2*(c%4)+1}. It loads the
    FULL x of its batch (8.4MB), computes LN stats locally (no
    AllReduce!), transposes x on the PE (needed for QKV anyway; the
    stats summation rides on ones-matmuls into PSUM), and projects
    q/k/v for its two heads over all 2048 tokens from a 384-column
    weight slice. Attention then needs NO resharding at all.
  - sims pack the two heads onto PE row halves (head0 rows 0..63,
    head1 rows 64..127) so both stream concurrently; exp on ScalarE
    covers both heads per [128,1024] tile; PV accumulates per head
    with a ones row for the softmax denominator.
  - output: one small AllToAll per 512-query i-block within the 4-core
    batch group (fires as soon as that i-block finishes, fully hidden
    under the next i-block's attention); postprocess (rownorm + out
    projection) trickles into PE slack between sim/PV work.
  - a tiny dummy AllReduce fires at t~0 so the one-time global comm
    rendezvous (~44us) runs concurrent with the whole local prologue.

Numerics: all matmuls bf16 with fp32 PSUM accumulation.

The kernel is self-contained: shapes are hardcoded to the problem spec.
"""

import numpy as np

# -------- problem constants (hardcoded per spec) --------
B = 2
NSEQ = 2048   # sequence length per batch (= tokens per core)
DIM = 1024
HEADS = 8
DHEAD = 64
INNER = HEADS * DHEAD  # 512
EPS = 1e-5
NCORES = 8
P = 128

SCALE = DHEAD ** -0.5  # 0.125
DC = DIM // P          # 8 d-model chunks
XT = NSEQ // P         # 16 x token-subtiles per core
IB = NSEQ // 512       # 4 i-blocks of 512 queries
JC = NSEQ // P         # 16 j-chunks of 128 keys
DE = DHEAD + 1         # head dims + denominator row
W2C = 6 * DHEAD        # 384 projection columns per core (k0 k1 q0 q1 v0 v1)
GS = NCORES // B       # 4 cores per batch group
TB = 512 // GS         # 128 tokens per core per i-block after out-A2A


def build_body(tc, outs, ins, dbg=False):
    """Emit the per-core program. outs/ins are dicts of DRAM APs."""
    import concourse.mybir as mybir
    from concourse.masks import make_identity

    dt = mybir.dt
    AF = mybir.ActivationFunctionType
    ALU = mybir.AluOpType
    nc = tc.nc

    NTOK = float(NSEQ)
    RG = [list(range(NCORES))]
    RGB = [list(range(GS)), list(range(GS, NCORES))]

    x = ins["x"]      # [NSEQ, DIM] f32 (this core's full batch)
    g = ins["g"]      # [DIM] f32
    w2 = ins["w2"]    # [DIM, 384] f32: columns [k_h0 k_h1 q_h0 q_h1 v_h0 v_h1]
    wo = ins["wo"]    # [INNER, DIM] f32 (replicated)
    out = outs["out"]  # [IB*TB, DIM] f32 (rows: ib*TB + t)

    with (
        tc.tile_pool(name="persist", bufs=1) as pp,
        tc.tile_pool(name="work", bufs=3) as pool,
        tc.tile_pool(name="work2", bufs=2) as pool2,
        tc.tile_pool(name="pref", bufs=6) as pref,
        tc.tile_pool(name="psA", bufs=2, space="PSUM") as psA,
        tc.tile_pool(name="psB", bufs=2, space="PSUM") as psB,
        tc.tile_pool(name="psC", bufs=1, space="PSUM") as psC,
        tc.tile_pool(name="dram", bufs=1, space="DRAM") as dram,
    ):
        # -------- dummy rendezvous collective (absorbs launch skew /
        # one-time comm setup while the local prologue runs)
        dummy_in = dram.tile([1, 8], dt.float32, tag="dmyi")
        dummy_out = dram.tile([1, 8], dt.float32, tag="dmyo")
        nc.gpsimd.collective_compute(
            "AllReduce", ALU.add, replica_groups=RG,
            ins=[dummy_in.opt()], outs=[dummy_out.opt()],
        )

        # -------- constants
        ident = pp.tile([P, P], dt.bfloat16)
        make_identity(nc, ident)
        g_sb = pp.tile([P, DC], dt.float32)
        nc.sync.dma_start(g_sb[:], g.rearrange("(c p) -> p c", p=P))
        ones1 = pp.tile([P, 1], dt.bfloat16)
        nc.vector.memset(ones1[:], 1.0)
        # head-broadcast selector for the rownorm: sel[h, kc, m] = 1 iff
        # inner element kc*128+m belongs to head h (= 2*kc + (m >= 64))
        sel_np = np.zeros((HEADS, INNER // P, P), np.float32)
        for kc in range(INNER // P):
            sel_np[2 * kc, kc, 0:DHEAD] = 1.0
            sel_np[2 * kc + 1, kc, DHEAD:P] = 1.0
        sel_dram = nc.inline_tensor(sel_np, name="selmat")
        sel = pp.tile([HEADS, INNER // P, P], dt.float32)
        nc.sync.dma_start(sel[:], sel_dram.ap())

        # -------- weights: w2 slice on the scalar queue (needed at QKV
        # start); wo afterwards (needed only by the out-projection)
        w2_bf = pp.tile([P, DC, W2C], dt.bfloat16)
        for kc in range(DC):
            wl = pool.tile([P, W2C], dt.float32, tag="wload")
            nc.scalar.dma_start(wl[:], w2[kc * P:(kc + 1) * P, :])
            nc.vector.tensor_copy(w2_bf[:, kc, :], wl[:])
        wo_bf = pp.tile([P, INNER // P, DIM], dt.bfloat16)
        for kc in range(INNER // P):
            wol = pool.tile([P, DIM], dt.float32, tag="wload")
            nc.scalar.dma_start(wol[:], wo[kc * P:(kc + 1) * P, :])
            nc.gpsimd.tensor_copy(wo_bf[:, kc, :], wol[:])

        # -------- phase 0: x load -> cast -> transpose -> xT; LN stats
        # accumulate in PSUM via ones-matmuls on the natural layout.
        xT = pp.tile([P, DC, NSEQ], dt.bfloat16)
        # stats accumulators: PSUM matmul outputs must sit at partition
        # 0/32/64, so (sum, sumsq) for each 512-col half live at rows 0
        # and 64 of two [P, 512] tiles.
        st_ps = [psC.tile([P, 512], dt.float32, tag=f"acc{h}",
                          name=f"st_ps{h}") for h in range(2)]
        for t in range(XT):
            x_f = pool.tile([P, DIM], dt.float32, tag="xload")
            nc.sync.dma_start(x_f[:], x[t * P:(t + 1) * P, :])
            x_bf = pool.tile([P, DIM], dt.bfloat16, tag="xbf")
            if t % 2 == 0:
                nc.vector.tensor_copy(x_bf[:], x_f[:])
            else:
                nc.gpsimd.tensor_copy(x_bf[:], x_f[:])
            sq = pool.tile([P, DIM], dt.bfloat16, tag="sq")
            nc.scalar.activation(sq[:], x_bf[:], AF.Square)
            for half in range(2):
                nc.tensor.matmul(
                    st_ps[half][0:1, :], ones1[:],
                    x_bf[:, half * 512:(half + 1) * 512],
                    start=(t == 0), stop=(t == XT - 1),
                )
                nc.tensor.matmul(
                    st_ps[half][DHEAD:DHEAD + 1, :], ones1[:],
                    sq[:, half * 512:(half + 1) * 512],
                    start=(t == 0), stop=(t == XT - 1),
                )
            tp = psB.tile([P, DIM], dt.bfloat16, tag="tr")
            for dc in range(DC):
                nc.tensor.transpose(
                    tp[:, dc * P:(dc + 1) * P],
                    x_bf[:, dc * P:(dc + 1) * P],
                    ident[:],
                )
            nc.vector.tensor_copy(
                xT[:, :, t * P:(t + 1) * P],
                tp[:].rearrange("p (dc j) -> p dc j", dc=DC),
            )

        # -------- LN coefficients (local, no collective!)
        # free->partition reshape must bounce through DRAM (SBUF APs
        # can't step partitions through free memory; PSUM can't feed
        # DMA directly, so hop PSUM -> SBUF -> DRAM -> stats).
        st_sb = [pool.tile([P, 512], dt.float32, tag="stc",
                           name=f"st_sb{h}") for h in range(2)]
        for h in range(2):
            nc.vector.tensor_copy(st_sb[h][:], st_ps[h][:])
        st_dram = dram.tile([4, 512], dt.float32, tag="stdr")
        for h in range(2):
            nc.sync.dma_start(st_dram[h], st_sb[h][0:1, :])
            nc.sync.dma_start(st_dram[2 + h],
                              st_sb[h][DHEAD:DHEAD + 1, :])
        stats = pp.tile([P, 2 * DC], dt.float32)
        # stats[p, dc] = sum[dc*128+p]; stats[p, DC+dc] = sumsq[dc*128+p]
        nc.sync.dma_start(
            stats[:, 0:DC],
            st_dram[0:2].rearrange("h (q p) -> p (h q)", p=P),
        )
        nc.sync.dma_start(
            stats[:, DC:2 * DC],
            st_dram[2:4].rearrange("h (q p) -> p (h q)", p=P),
        )
        mean = pp.tile([P, DC], dt.float32)
        nc.vector.tensor_scalar_mul(mean[:], stats[:, 0:DC], 1.0 / NTOK)
        e2 = pp.tile([P, DC], dt.float32)
        nc.vector.tensor_scalar_mul(e2[:], stats[:, DC:2 * DC], 1.0 / NTOK)
        msq = pp.tile([P, DC], dt.float32)
        nc.vector.tensor_tensor(msq[:], mean[:], mean[:], ALU.mult)
        vareps = pp.tile([P, DC], dt.float32)
        nc.vector.tensor_tensor(vareps[:], e2[:], msq[:], ALU.subtract)
        nc.vector.tensor_scalar_add(vareps[:], vareps[:], EPS)
        rvar = pp.tile([P, DC], dt.float32)
        nc.vector.reciprocal(rvar[:], vareps[:])
        rstd = pp.tile([P, DC], dt.float32)
        nc.scalar.activation(rstd[:], rvar[:], AF.Sqrt)
        A2 = pp.tile([P, DC], dt.float32)
        nc.vector.tensor_tensor(A2[:], rstd[:], g_sb[:], ALU.mult)
        C2 = pp.tile([P, DC], dt.float32)
        nc.vector.tensor_tensor(C2[:], mean[:], A2[:], ALU.mult)
        nc.vector.tensor_scalar_mul(C2[:], C2[:], -1.0)
        # preload the exp table while the PE chews on QKV
        junk = pp.tile([1, DC], dt.float32)
        nc.scalar.activation(junk[:], A2[0:1, :], AF.Exp)
        if dbg:
            nc.sync.dma_start(outs["dbg_stats"], stats[:])
            nc.sync.dma_start(outs["dbg_A2"], A2[:])
            nc.sync.dma_start(outs["dbg_C2"], C2[:])

        # normalize xT in place
        for dc in range(DC):
            nc.vector.tensor_scalar(
                xT[:, dc, :], xT[:, dc, :],
                A2[:, dc:dc + 1], C2[:, dc:dc + 1],
                ALU.mult, ALU.add,
            )

        # -------- QKV projection for this core's 2 heads over all tokens.
        # w2 columns: [k(128) | q(128) | v(128)], head-major inside each.
        # k/q land transposed ([dims, tokens]) which is exactly the sim
        # layout; v lands as vT and is flipped by 32 tiny DMA transposes.
        kTh = pp.tile([P, NSEQ], dt.bfloat16)
        qTh = pp.tile([P, NSEQ], dt.bfloat16)
        vT = pp.tile([P, NSEQ], dt.bfloat16)
        dsts = [kTh, qTh, vT]
        for blk in range(3):
            for tp2 in range(2):
                ps = psA.tile([P, 1024], dt.float32, tag="sim")
                for half in range(2):
                    tcn = tp2 * 2 + half
                    for kc in range(DC):
                        nc.tensor.matmul(
                            ps[:, half * 512:(half + 1) * 512],
                            w2_bf[:, kc, blk * P:(blk + 1) * P],
                            xT[:, kc, tcn * 512:(tcn + 1) * 512],
                            start=(kc == 0), stop=(kc == DC - 1),
                        )
                nc.vector.tensor_copy(
                    dsts[blk][:, tp2 * 1024:(tp2 + 1) * 1024], ps[:])

        # vext[h][j, jc, d|1]: per-head value tiles with the ones column
        vext = [pp.tile([P, JC, DE], dt.bfloat16, name=f"vext{h}")
                for h in range(2)]
        for h in range(2):
            nc.gpsimd.memset(vext[h][:, :, DHEAD:DE], 1.0)
        for tv in range(JC):
            vtp = psB.tile([P, P], dt.bfloat16, tag="tr")
            nc.tensor.transpose(vtp[:], vT[:, tv * P:(tv + 1) * P],
                                ident[:])
            for h in range(2):
                nc.vector.tensor_copy(vext[h][:, tv, 0:DHEAD],
                                      vtp[:, h * DHEAD:(h + 1) * DHEAD])
        if dbg:
            nc.sync.dma_start(outs["dbg_xn"], xT[:])
            nc.sync.dma_start(outs["dbg_kT"], kTh[:])
            nc.sync.dma_start(outs["dbg_qT"], qTh[:])
            nc.sync.dma_start(outs["dbg_v0"], vext[0][:])
            nc.sync.dma_start(outs["dbg_v1"], vext[1][:])

        # -------- attention + per-i-block out A2A, postprocess trickled
        def attn_block(ib, todo=()):
            todo = list(todo)
            i0 = ib * 512
            ot = [psC.tile([DE, 512], dt.float32, tag=f"acc{h}",
                          name=f"ot{h}") for h in range(2)]
            for jc in range(JC):
                if jc % 4 == 3 and todo:
                    todo.pop(0)()
                sp = psA.tile([P, 1024], dt.float32, tag="sim")
                for h in range(2):
                    rsl = slice(h * DHEAD, (h + 1) * DHEAD)
                    nc.tensor.matmul(
                        sp[:, h * 512:(h + 1) * 512],
                        kTh[rsl, jc * P:(jc + 1) * P],
                        qTh[rsl, i0:i0 + 512],
                        start=True, stop=True,
                    )
                et = pref.tile([P, 1024], dt.bfloat16, tag="exp")
                nc.scalar.activation(et[:], sp[:], AF.Exp, scale=SCALE)
                for h in range(2):
                    nc.tensor.matmul(
                        ot[h][:], vext[h][:, jc, :],
                        et[:, h * 512:(h + 1) * 512],
                        start=(jc == 0), stop=(jc == JC - 1),
                    )
            ao = pool2.tile([DE, 2, 512], dt.bfloat16, tag="ao")
            for h in range(2):
                nc.vector.tensor_copy(ao[:, h, :], ot[h][:])
            return ao

        def out_a2a(ib, ao):
            # all-8 AllToAll: destination core d gets tokens
            # [ib*512 + d*64, +64) of BOTH batches (rows 0..3 = batch-0
            # heads, rows 4..7 = batch-1 heads) -- mesh needs >4 cores.
            TH = TB // 2
            a2a_in = dram.tile([NCORES, 2, DE, TH], dt.bfloat16,
                               tag=f"oa{ib}")
            for h in range(2):
                nc.sync.dma_start(
                    a2a_in[:, h].rearrange("r d t -> d r t"),
                    ao[:, h, :].rearrange("d (r t) -> d r t", r=NCORES),
                )
            a2a_out = dram.tile([NCORES, 2, DE, TH], dt.bfloat16,
                                tag=f"ob{ib}")
            nc.gpsimd.collective_compute(
                "AllToAll", ALU.bypass, replica_groups=RG,
                ins=[a2a_in.opt()], outs=[a2a_out.opt()],
            )
            return a2a_out

        def pp_stages(ib, a2a_out):
            """Postprocess one i-block's received tokens (64 per batch,
            packed side by side into 128 columns), split into stages so
            the PE work trickles into attention slack."""
            st = {}
            TH = TB // 2

            def s_gather():
                # ao_g[hh*64+d, s, sb*64+t] = a2a_out[sb*4+s, hh, d, t]
                ao_g = pool2.tile([P, GS, TB], dt.bfloat16, tag="aog")
                for hh in range(2):
                    for sb in range(2):
                        nc.sync.dma_start(
                            ao_g[hh * DHEAD:(hh + 1) * DHEAD, :,
                                 sb * TH:(sb + 1) * TH],
                            a2a_out[sb * GS:(sb + 1) * GS, hh, 0:DHEAD,
                                    :].rearrange("s d t -> d s t"),
                        )
                rn = pool2.tile([HEADS, TB], dt.bfloat16, tag="rn")
                for sb in range(2):
                    nc.sync.dma_start(
                        rn[:, sb * TH:(sb + 1) * TH],
                        a2a_out[sb * GS:(sb + 1) * GS, :, DHEAD,
                                :].rearrange("s h t -> (s h) t"),
                    )
                rc = pool2.tile([HEADS, TB], dt.float32, tag="rc")
                nc.vector.reciprocal(rc[:], rn[:])
                st["ao_g"], st["rc"] = ao_g, rc

            def s_norm():
                ao_g, rc = st["ao_g"], st["rc"]
                for kc in range(INNER // P):
                    bcp = psB.tile([P, TB], dt.float32, tag="tr")
                    nc.tensor.matmul(bcp[:], sel[:, kc, :], rc[:],
                                     start=True, stop=True)
                    nc.vector.tensor_tensor(
                        ao_g[:, kc, :], ao_g[:, kc, :], bcp[:], ALU.mult
                    )

            def s_proj(nh2):
                ao_g = st["ao_g"]
                op = psB.tile([P, 512], dt.float32, tag="tr")
                for kc in range(INNER // P):
                    nc.tensor.matmul(
                        op[:], ao_g[:, kc, :],
                        wo_bf[:, kc, nh2 * 512:(nh2 + 1) * 512],
                        start=(kc == 0), stop=(kc == INNER // P - 1),
                    )
                out_sb = st.setdefault(
                    "osb", pool2.tile([P, DIM], dt.float32, tag="osb",
                                      name="out_sb"))
                nc.vector.tensor_copy(
                    out_sb[:, nh2 * 512:(nh2 + 1) * 512], op[:])
                if nh2 == DIM // 512 - 1:
                    # rows 0..63 = batch-0 tokens, 64..127 = batch-1
                    nc.sync.dma_start(out[ib * TB:(ib + 1) * TB, :],
                                      out_sb[:])

            return [s_gather, s_norm] + \
                [lambda nh2=nh2: s_proj(nh2) for nh2 in range(DIM // 512)]

        # each i-block's pp stages run interleaved inside the NEXT block's
        # jc stream (emission order guides the Tile scheduler; data deps
        # keep everything correct); the last block's pp is the only tail.
        pend = []
        for ib in range(IB):
            ao = attn_block(ib, pend)
            pend = []
            o = out_a2a(ib, ao)
            pend.extend(pp_stages(ib, o))
        for s in pend:
            s()


def build_graph(dbg=False):
    import concourse.mybir as mybir
    import concourse.tile as tile
    from concourse import bacc

    dt = mybir.dt
    nc = bacc.Bacc("TRN2", target_bir_lowering=False, debug=False,
                   num_devices=NCORES)
    ins = {
        "x": nc.dram_tensor("x", [NSEQ, DIM], dt.float32,
                            kind="ExternalInput").ap(),
        "g": nc.dram_tensor("g", [DIM], dt.float32,
                            kind="ExternalInput").ap(),
        "w2": nc.dram_tensor("w2", [DIM, W2C], dt.float32,
                             kind="ExternalInput").ap(),
        "wo": nc.dram_tensor("wo", [INNER, DIM], dt.float32,
                             kind="ExternalInput").ap(),
    }
    outs = {
        "out": nc.dram_tensor("out", [IB * TB, DIM], dt.float32,
                              kind="ExternalOutput").ap(),
    }
    if dbg:
        for name, shape, dt_ in (
            ("dbg_stats", [P, 2 * DC], dt.float32),
            ("dbg_A2", [P, DC], dt.float32),
            ("dbg_C2", [P, DC], dt.float32),
            ("dbg_xn", [P, DC, NSEQ], dt.bfloat16),
            ("dbg_kT", [P, NSEQ], dt.bfloat16),
            ("dbg_qT", [P, NSEQ], dt.bfloat16),
            ("dbg_v0", [P, JC, DE], dt.bfloat16),
            ("dbg_v1", [P, JC, DE], dt.bfloat16),
        ):
            outs[name] = nc.dram_tensor(name, shape, dt_,
                                        kind="ExternalOutput").ap()
    with tile.TileContext(nc) as tc:
        build_body(tc, outs, ins, dbg=dbg)
    nc.compile()
    return nc


def make_in_maps(x, g, wq, wkv, wo):
    """Shard full inputs into per-core input maps."""
    x_ = np.asarray(x, np.float32)
    g_ = np.ascontiguousarray(np.asarray(g, np.float32))
    wq_ = np.asarray(wq, np.float32)
    wkv_ = np.asarray(wkv, np.float32)
    wo_ = np.ascontiguousarray(np.asarray(wo, np.float32))
    wk_ = wkv_[:, :INNER]
    wv_ = wkv_[:, INNER:]
    in_maps = []
    for c in range(NCORES):
        b, r = divmod(c, GS)
        h0 = 2 * r * DHEAD
        h2 = h0 + 2 * DHEAD
        w2 = np.ascontiguousarray(np.concatenate(
            [wk_[:, h0:h2], wq_[:, h0:h2], wv_[:, h0:h2]], axis=1))
        in_maps.append({
            "x": np.ascontiguousarray(x_[b]),
            "g": g_,
            "w2": w2,
            "wo": wo_,
        })
    return in_maps


def assemble_out(core_outs):
    """core c, row ib*128 + sb*64 + t -> token sb*NSEQ + ib*512 + c*64 + t."""
    TH = TB // 2
    full = np.empty((B * NSEQ, DIM), np.float32)
    for c in range(NCORES):
        o = core_outs[c]
        for ib in range(IB):
            for sb in range(B):
                dst = sb * NSEQ + ib * 512 + c * TH
                src_r = ib * TB + sb * TH
                full[dst:dst + TH] = o[src_r:src_r + TH]
    return full


_cache = {}


def _get_graph():
    if "nc" not in _cache:
        _cache["nc"] = build_graph()
    return _cache["nc"]


def run_on_hw(in_maps, trace=False, **kw):
    from concourse.bass_utils import run_bass_kernel_spmd
    nc = _get_graph()
    return run_bass_kernel_spmd(
        nc, in_maps, core_ids=list(range(NCORES)), trace=trace, **kw
    )


def kernel(x, g, wq, wkv, wo):
    in_maps = make_in_maps(x, g, wq, wkv, wo)
    res = run_on_hw(in_maps)
    core_outs = [np.asarray(res.results[c]["out"], np.float32)
                 for c in range(NCORES)]
    return assemble_out(core_outs).reshape(B, NSEQ, DIM)


# revision 21
# speedup vs baseline: 1.0850x; 1.0551x over previous
"""Distributed Trainium2 kernel for nn_Attention_29832842838194.

LayerNorm (stats over the *sequence* axis) -> QKV projection -> 8-head
attention (N=2048, d_head=64) -> output projection, on 8 NeuronCores.

Sharding (v3 — one batch + two heads per core, collective-free front):
  - core c owns batch c//4 and heads {2*(c%4), 2*(c%4)+1}. It loads the
    FULL x of its batch (8.4MB), computes LN stats locally (no
    AllReduce!), transposes x on the PE (needed for QKV anyway; the
    stats summation rides on ones-matmuls into PSUM), and projects
    q/k/v for its two heads over all 2048 tokens from a 384-column
    weight slice. Attention then needs NO resharding at all.
  - sims pack the two heads onto PE row halves (head0 rows 0..63,
    head1 rows 64..127) so both stream concurrently; exp on ScalarE
    covers both heads per [128,1024] tile; PV accumulates per head
    with a ones row for the softmax denominator.
  - output: one small AllToAll per 512-query i-block within the 4-core
    batch group (fires as soon as that i-block finishes, fully hidden
    under the next i-block's attention); postprocess (rownorm + out
    projection) trickles into PE slack between sim/PV work.
  - a tiny dummy AllReduce fires at t~0 so the one-time global comm
    rendezvous (~44us) runs concurrent with the whole local prologue.

Numerics: all matmuls bf16 with fp32 PSUM accumulation.

The kernel is self-contained: shapes are hardcoded to the problem spec.
"""

import numpy as np

# -------- problem constants (hardcoded per spec) --------
B = 2
NSEQ = 2048   # sequence length per batch (= tokens per core)
DIM = 1024
HEADS = 8
DHEAD = 64
INNER = HEADS * DHEAD  # 512
EPS = 1e-5
NCORES = 8
P = 128

SCALE = DHEAD ** -0.5  # 0.125
DC = DIM // P          # 8 d-model chunks
XT = NSEQ // P         # 16 x token-subtiles per core
IB = NSEQ // 512       # 4 i-blocks of 512 queries
JC = NSEQ // P         # 16 j-chunks of 128 keys
DE = DHEAD + 1         # head dims + denominator row
W2C = 6 * DHEAD        # 384 projection columns per core (k0 k1 q0 q1 v0 v1)
GS = NCORES // B       # 4 cores per batch group
TB = 512 // GS         # 128 tokens per core per i-block after out-A2A


def build_body(tc, outs, ins, dbg=False):
    """Emit the per-core program. outs/ins are dicts of DRAM APs."""
    import concourse.mybir as mybir
    from concourse.masks import make_identity

    dt = mybir.dt
    AF = mybir.ActivationFunctionType
    ALU = mybir.AluOpType
    nc = tc.nc

    NTOK = float(NSEQ)
    RG = [list(range(NCORES))]
    RGB = [list(range(GS)), list(range(GS, NCORES))]

    x = ins["x"]      # [NSEQ, DIM] f32 (this core's full batch)
    g = ins["g"]      # [DIM] f32
    w2 = ins["w2"]    # [DIM, 384] f32: columns [k_h0 k_h1 q_h0 q_h1 v_h0 v_h1]
    wo = ins["wo"]    # [INNER, DIM] f32 (replicated)
    out = outs["out"]  # [IB*TB, DIM] f32 (rows: ib*TB + t)

    with (
        tc.tile_pool(name="persist", bufs=1) as pp,
        tc.tile_pool(name="work", bufs=3) as pool,
        tc.tile_pool(name="work2", bufs=2) as pool2,
        tc.tile_pool(name="pref", bufs=6) as pref,
        tc.tile_pool(name="psA", bufs=2, space="PSUM") as psA,
        tc.tile_pool(name="psB", bufs=2, space="PSUM") as psB,
        tc.tile_pool(name="psC", bufs=1, space="PSUM") as psC,
        tc.tile_pool(name="dram", bufs=1, space="DRAM") as dram,
    ):
        # -------- dummy rendezvous collective (absorbs launch skew /
        # one-time comm setup while the local prologue runs)
        dummy_in = dram.tile([1, 8], dt.float32, tag="dmyi")
        dummy_out = dram.tile([1, 8], dt.float32, tag="dmyo")
        nc.gpsimd.collective_compute(
            "AllReduce", ALU.add, replica_groups=RG,
            ins=[dummy_in.opt()], outs=[dummy_out.opt()],
        )

        # -------- constants
        ident = pp.tile([P, P], dt.bfloat16)
        make_identity(nc, ident)
        g_sb = pp.tile([P, DC], dt.float32)
        nc.sync.dma_start(g_sb[:], g.rearrange("(c p) -> p c", p=P))
        ones1 = pp.tile([P, 1], dt.bfloat16)
        nc.vector.memset(ones1[:], 1.0)
        # head-broadcast selector for the rownorm: sel[h, kc, m] = 1 iff
        # inner element kc*128+m belongs to head h (= 2*kc + (m >= 64))
        sel_np = np.zeros((HEADS, INNER // P, P), np.float32)
        for kc in range(INNER // P):
            sel_np[2 * kc, kc, 0:DHEAD] = 1.0
            sel_np[2 * kc + 1, kc, DHEAD:P] = 1.0
        sel_dram = nc.inline_tensor(sel_np, name="selmat")
        sel = pp.tile([HEADS, INNER // P, P], dt.float32)
        nc.sync.dma_start(sel[:], sel_dram.ap())

        # -------- weights: w2 slice early on the scalar queue (needed at
        # QKV start); wo is loaded after the x stream (needed only by the
        # out-projection, ~40us later)
        w2_bf = pp.tile([P, DC, W2C], dt.bfloat16)
        for kc in range(DC):
            wl = pool.tile([P, W2C], dt.float32, tag="wload")
            nc.scalar.dma_start(wl[:], w2[kc * P:(kc + 1) * P, :])
            nc.vector.tensor_copy(w2_bf[:, kc, :], wl[:])
        wo_bf = pp.tile([P, INNER // P, DIM], dt.bfloat16)

        # -------- phase 0: x load -> cast -> transpose -> xT; LN stats
        # accumulate in PSUM via ones-matmuls on the natural layout.
        xT = pp.tile([P, DC, NSEQ], dt.bfloat16)
        # stats accumulators: PSUM matmul outputs must sit at partition
        # 0/32/64, so (sum, sumsq) for each 512-col half live at rows 0
        # and 64 of two [P, 512] tiles.
        st_ps = [psC.tile([P, 512], dt.float32, tag=f"acc{h}",
                          name=f"st_ps{h}") for h in range(2)]
        for t in range(XT):
            x_f = pool.tile([P, DIM], dt.float32, tag="xload")
            (nc.sync if t % 2 == 0 else nc.scalar).dma_start(
                x_f[:], x[t * P:(t + 1) * P, :])
            x_bf = pool.tile([P, DIM], dt.bfloat16, tag="xbf")
            if t % 2 == 0:
                nc.vector.tensor_copy(x_bf[:], x_f[:])
            else:
                nc.scalar.copy(x_bf[:], x_f[:])
            # squares on the DVE (bf16 2x mode beats ACT here)
            sq = pool.tile([P, DIM], dt.bfloat16, tag="sq")
            nc.vector.tensor_tensor(sq[:], x_bf[:], x_bf[:], ALU.mult)
            for half in range(2):
                nc.tensor.matmul(
                    st_ps[half][0:1, :], ones1[:],
                    x_bf[:, half * 512:(half + 1) * 512],
                    start=(t == 0), stop=(t == XT - 1),
                )
                nc.tensor.matmul(
                    st_ps[half][DHEAD:DHEAD + 1, :], ones1[:],
                    sq[:, half * 512:(half + 1) * 512],
                    start=(t == 0), stop=(t == XT - 1),
                )
            tp = psB.tile([P, DIM], dt.bfloat16, tag="tr")
            for dc in range(DC):
                nc.tensor.transpose(
                    tp[:, dc * P:(dc + 1) * P],
                    x_bf[:, dc * P:(dc + 1) * P],
                    ident[:],
                )
            xdst = xT[:, :, t * P:(t + 1) * P]
            tsrc = tp[:].rearrange("p (dc j) -> p dc j", dc=DC)
            if t % 2 == 0:
                nc.vector.tensor_copy(xdst, tsrc)
            else:
                nc.scalar.copy(xdst, tsrc)
        # wo loads go behind the x stream on the scalar queue
        for kc in range(INNER // P):
            wol = pool.tile([P, DIM], dt.float32, tag="wload")
            nc.scalar.dma_start(wol[:], wo[kc * P:(kc + 1) * P, :])
            nc.gpsimd.tensor_copy(wo_bf[:, kc, :], wol[:])

        # -------- LN coefficients (local, no collective!)
        # free->partition reshape must bounce through DRAM (SBUF APs
        # can't step partitions through free memory; PSUM can't feed
        # DMA directly, so hop PSUM -> SBUF -> DRAM -> stats).
        st_sb = [pool.tile([P, 512], dt.float32, tag="stc",
                           name=f"st_sb{h}") for h in range(2)]
        for h in range(2):
            nc.vector.tensor_copy(st_sb[h][:], st_ps[h][:])
        st_dram = dram.tile([4, 512], dt.float32, tag="stdr")
        for h in range(2):
            nc.sync.dma_start(st_dram[h], st_sb[h][0:1, :])
            nc.sync.dma_start(st_dram[2 + h],
                              st_sb[h][DHEAD:DHEAD + 1, :])
        stats = pp.tile([P, 2 * DC], dt.float32)
        # stats[p, dc] = sum[dc*128+p]; stats[p, DC+dc] = sumsq[dc*128+p]
        nc.sync.dma_start(
            stats[:, 0:DC],
            st_dram[0:2].rearrange("h (q p) -> p (h q)", p=P),
        )
        nc.sync.dma_start(
            stats[:, DC:2 * DC],
            st_dram[2:4].rearrange("h (q p) -> p (h q)", p=P),
        )
        mean = pp.tile([P, DC], dt.float32)
        nc.vector.tensor_scalar_mul(mean[:], stats[:, 0:DC], 1.0 / NTOK)
        e2 = pp.tile([P, DC], dt.float32)
        nc.vector.tensor_scalar_mul(e2[:], stats[:, DC:2 * DC], 1.0 / NTOK)
        msq = pp.tile([P, DC], dt.float32)
        nc.vector.tensor_tensor(msq[:], mean[:], mean[:], ALU.mult)
        vareps = pp.tile([P, DC], dt.float32)
        nc.vector.tensor_tensor(vareps[:], e2[:], msq[:], ALU.subtract)
        nc.vector.tensor_scalar_add(vareps[:], vareps[:], EPS)
        rvar = pp.tile([P, DC], dt.float32)
        nc.vector.reciprocal(rvar[:], vareps[:])
        rstd = pp.tile([P, DC], dt.float32)
        nc.scalar.activation(rstd[:], rvar[:], AF.Sqrt)
        A2 = pp.tile([P, DC], dt.float32)
        nc.vector.tensor_tensor(A2[:], rstd[:], g_sb[:], ALU.mult)
        C2 = pp.tile([P, DC], dt.float32)
        nc.vector.tensor_tensor(C2[:], mean[:], A2[:], ALU.mult)
        nc.vector.tensor_scalar_mul(C2[:], C2[:], -1.0)
        # preload the exp table while the PE chews on QKV
        junk = pp.tile([1, DC], dt.float32)
        nc.scalar.activation(junk[:], A2[0:1, :], AF.Exp)
        if dbg:
            nc.sync.dma_start(outs["dbg_stats"], stats[:])
            nc.sync.dma_start(outs["dbg_A2"], A2[:])
            nc.sync.dma_start(outs["dbg_C2"], C2[:])

        # fold LN into the projection: q = x @ (A*w2) + (-mean) @ (A*w2),
        # so xT stays RAW and the per-token normalize pass disappears.
        for kc in range(DC):
            nc.vector.tensor_scalar(
                w2_bf[:, kc, :], w2_bf[:, kc, :],
                A2[:, kc:kc + 1], None, ALU.mult,
            )
        negmu = pp.tile([P, DC], dt.bfloat16)
        nc.vector.tensor_scalar(negmu[:], mean[:], -1.0, None, ALU.mult)
        bp = psB.tile([1, W2C], dt.float32, tag="tr")
        for kc in range(DC):
            nc.tensor.matmul(
                bp[:], negmu[:, kc:kc + 1], w2_bf[:, kc, :],
                start=(kc == 0), stop=(kc == DC - 1),
            )
        bp_sb = pool.tile([1, W2C], dt.float32, tag="stc")
        nc.vector.tensor_copy(bp_sb[:], bp[:])
        bias_dram = dram.tile([1, W2C], dt.float32, tag="biasd")
        nc.sync.dma_start(bias_dram[:], bp_sb[:])
        bias_sb = pp.tile([P, 3], dt.float32)
        nc.sync.dma_start(
            bias_sb[:],
            bias_dram[:].rearrange("o (c p) -> p (o c)", p=P),
        )

        # -------- QKV projection for this core's 2 heads over all tokens.
        # w2 columns: [k(128) | q(128) | v(128)], head-major inside each.
        # k/q land transposed ([dims, tokens]) which is exactly the sim
        # layout; v lands as vT and is flipped by 32 tiny DMA transposes.
        kTh = pp.tile([P, NSEQ], dt.bfloat16)
        qTh = pp.tile([P, NSEQ], dt.bfloat16)
        vT = pp.tile([P, NSEQ], dt.bfloat16)
        dsts = [kTh, qTh, vT]
        for blk in range(3):
            for tp2 in range(2):
                ps = psA.tile([P, 1024], dt.float32, tag="sim")
                for half in range(2):
                    tcn = tp2 * 2 + half
                    for kc in range(DC):
                        nc.tensor.matmul(
                            ps[:, half * 512:(half + 1) * 512],
                            w2_bf[:, kc, blk * P:(blk + 1) * P],
                            xT[:, kc, tcn * 512:(tcn + 1) * 512],
                            start=(kc == 0), stop=(kc == DC - 1),
                        )
                nc.vector.tensor_scalar(
                    dsts[blk][:, tp2 * 1024:(tp2 + 1) * 1024], ps[:],
                    bias_sb[:, blk:blk + 1], None, ALU.add,
                )

        # vext[h][j, jc, d|1]: per-head value tiles with the ones column
        vext = [pp.tile([P, JC, DE], dt.bfloat16, name=f"vext{h}")
                for h in range(2)]
        for h in range(2):
            nc.gpsimd.memset(vext[h][:, :, DHEAD:DE], 1.0)
        for tv in range(JC):
            vtp = psB.tile([P, P], dt.bfloat16, tag="tr")
            nc.tensor.transpose(vtp[:], vT[:, tv * P:(tv + 1) * P],
                                ident[:])
            for h in range(2):
                nc.vector.tensor_copy(vext[h][:, tv, 0:DHEAD],
                                      vtp[:, h * DHEAD:(h + 1) * DHEAD])
        if dbg:
            nc.sync.dma_start(outs["dbg_xn"], xT[:])
            nc.sync.dma_start(outs["dbg_kT"], kTh[:])
            nc.sync.dma_start(outs["dbg_qT"], qTh[:])
            nc.sync.dma_start(outs["dbg_v0"], vext[0][:])
            nc.sync.dma_start(outs["dbg_v1"], vext[1][:])

        # -------- attention + per-i-block out A2A, postprocess trickled
        def attn_block(ib, todo=()):
            todo = list(todo)
            i0 = ib * 512
            ot = [psC.tile([DE, 512], dt.float32, tag=f"acc{h}",
                          name=f"ot{h}") for h in range(2)]
            for jc in range(JC):
                if jc % 4 == 3 and todo:
                    todo.pop(0)()
                sp = psA.tile([P, 1024], dt.float32, tag="sim")
                for h in range(2):
                    rsl = slice(h * DHEAD, (h + 1) * DHEAD)
                    nc.tensor.matmul(
                        sp[:, h * 512:(h + 1) * 512],
                        kTh[rsl, jc * P:(jc + 1) * P],
                        qTh[rsl, i0:i0 + 512],
                        start=True, stop=True,
                    )
                et = pref.tile([P, 1024], dt.bfloat16, tag="exp")
                nc.scalar.activation(et[:], sp[:], AF.Exp, scale=SCALE)
                for h in range(2):
                    nc.tensor.matmul(
                        ot[h][:], vext[h][:, jc, :],
                        et[:, h * 512:(h + 1) * 512],
                        start=(jc == 0), stop=(jc == JC - 1),
                    )
            ao = pool2.tile([DE, 2, 512], dt.bfloat16, tag="ao")
            for h in range(2):
                nc.vector.tensor_copy(ao[:, h, :], ot[h][:])
            return ao

        def out_a2a(ib, ao):
            # all-8 AllToAll: destination core d gets tokens
            # [ib*512 + d*64, +64) of BOTH batches (rows 0..3 = batch-0
            # heads, rows 4..7 = batch-1 heads) -- mesh needs >4 cores.
            TH = TB // 2
            a2a_in = dram.tile([NCORES, 2, DE, TH], dt.bfloat16,
                               tag=f"oa{ib}")
            for h in range(2):
                nc.sync.dma_start(
                    a2a_in[:, h].rearrange("r d t -> d r t"),
                    ao[:, h, :].rearrange("d (r t) -> d r t", r=NCORES),
                )
            a2a_out = dram.tile([NCORES, 2, DE, TH], dt.bfloat16,
                                tag=f"ob{ib}")
            nc.gpsimd.collective_compute(
                "AllToAll", ALU.bypass, replica_groups=RG,
                ins=[a2a_in.opt()], outs=[a2a_out.opt()],
            )
            return a2a_out

        def pp_stages(ib, a2a_out):
            """Postprocess one i-block's received tokens (64 per batch,
            packed side by side into 128 columns), split into stages so
            the PE work trickles into attention slack."""
            st = {}
            TH = TB // 2

            def s_gather():
                # ao_g[hh*64+d, s, sb*64+t] = a2a_out[sb*4+s, hh, d, t]
                ao_g = pool2.tile([P, GS, TB], dt.bfloat16, tag="aog")
                for hh in range(2):
                    for sb in range(2):
                        nc.sync.dma_start(
                            ao_g[hh * DHEAD:(hh + 1) * DHEAD, :,
                                 sb * TH:(sb + 1) * TH],
                            a2a_out[sb * GS:(sb + 1) * GS, hh, 0:DHEAD,
                                    :].rearrange("s d t -> d s t"),
                        )
                rn = pool2.tile([HEADS, TB], dt.bfloat16, tag="rn")
                for sb in range(2):
                    nc.sync.dma_start(
                        rn[:, sb * TH:(sb + 1) * TH],
                        a2a_out[sb * GS:(sb + 1) * GS, :, DHEAD,
                                :].rearrange("s h t -> (s h) t"),
                    )
                rc = pool2.tile([HEADS, TB], dt.float32, tag="rc")
                nc.vector.reciprocal(rc[:], rn[:])
                st["ao_g"], st["rc"] = ao_g, rc

            def s_norm():
                ao_g, rc = st["ao_g"], st["rc"]
                for kc in range(INNER // P):
                    bcp = psB.tile([P, TB], dt.float32, tag="tr")
                    nc.tensor.matmul(bcp[:], sel[:, kc, :], rc[:],
                                     start=True, stop=True)
                    nc.vector.tensor_tensor(
                        ao_g[:, kc, :], ao_g[:, kc, :], bcp[:], ALU.mult
                    )

            def s_proj(nh2):
                ao_g = st["ao_g"]
                op = psB.tile([P, 512], dt.float32, tag="tr")
                for kc in range(INNER // P):
                    nc.tensor.matmul(
                        op[:], ao_g[:, kc, :],
                        wo_bf[:, kc, nh2 * 512:(nh2 + 1) * 512],
                        start=(kc == 0), stop=(kc == INNER // P - 1),
                    )
                out_sb = st.setdefault(
                    "osb", pool2.tile([P, DIM], dt.float32, tag="osb",
                                      name="out_sb"))
                nc.vector.tensor_copy(
                    out_sb[:, nh2 * 512:(nh2 + 1) * 512], op[:])
                if nh2 == DIM // 512 - 1:
                    # rows 0..63 = batch-0 tokens, 64..127 = batch-1
                    nc.sync.dma_start(out[ib * TB:(ib + 1) * TB, :],
                                      out_sb[:])

            return [s_gather, s_norm] + \
                [lambda nh2=nh2: s_proj(nh2) for nh2 in range(DIM // 512)]

        # pp(ib) runs interleaved inside attn(ib+2)'s jc stream: by then
        # its A2A has certainly landed, so the PE queue never blocks on
        # unready collective data (engine queues execute in-order).
        stages = {}
        for ib in range(IB):
            todo = stages.pop(ib - 2, [])
            ao = attn_block(ib, todo)
            o = out_a2a(ib, ao)
            stages[ib] = pp_stages(ib, o)
        for ib in sorted(stages):
            for s in stages[ib]:
                s()


def build_graph(dbg=False):
    import concourse.mybir as mybir
    import concourse.tile as tile
    from concourse import bacc

    dt = mybir.dt
    nc = bacc.Bacc("TRN2", target_bir_lowering=False, debug=False,
                   num_devices=NCORES)
    ins = {
        "x": nc.dram_tensor("x", [NSEQ, DIM], dt.float32,
                            kind="ExternalInput").ap(),
        "g": nc.dram_tensor("g", [DIM], dt.float32,
                            kind="ExternalInput").ap(),
        "w2": nc.dram_tensor("w2", [DIM, W2C], dt.float32,
                             kind="ExternalInput").ap(),
        "wo": nc.dram_tensor("wo", [INNER, DIM], dt.float32,
                             kind="ExternalInput").ap(),
    }
    outs = {
        "out": nc.dram_tensor("out", [IB * TB, DIM], dt.float32,
                              kind="ExternalOutput").ap(),
    }
    if dbg:
        for name, shape, dt_ in (
            ("dbg_stats", [P, 2 * DC], dt.float32),
            ("dbg_A2", [P, DC], dt.float32),
            ("dbg_C2", [P, DC], dt.float32),
            ("dbg_xn", [P, DC, NSEQ], dt.bfloat16),
            ("dbg_kT", [P, NSEQ], dt.bfloat16),
            ("dbg_qT", [P, NSEQ], dt.bfloat16),
            ("dbg_v0", [P, JC, DE], dt.bfloat16),
            ("dbg_v1", [P, JC, DE], dt.bfloat16),
        ):
            outs[name] = nc.dram_tensor(name, shape, dt_,
                                        kind="ExternalOutput").ap()
    with tile.TileContext(nc) as tc:
        build_body(tc, outs, ins, dbg=dbg)
    nc.compile()
    return nc


def make_in_maps(x, g, wq, wkv, wo):
    """Shard full inputs into per-core input maps."""
    x_ = np.asarray(x, np.float32)
    g_ = np.ascontiguousarray(np.asarray(g, np.float32))
    wq_ = np.asarray(wq, np.float32)
    wkv_ = np.asarray(wkv, np.float32)
    wo_ = np.ascontiguousarray(np.asarray(wo, np.float32))
    wk_ = wkv_[:, :INNER]
    wv_ = wkv_[:, INNER:]
    in_maps = []
    for c in range(NCORES):
        b, r = divmod(c, GS)
        h0 = 2 * r * DHEAD
        h2 = h0 + 2 * DHEAD
        w2 = np.ascontiguousarray(np.concatenate(
            [wk_[:, h0:h2], wq_[:, h0:h2], wv_[:, h0:h2]], axis=1))
        in_maps.append({
            "x": np.ascontiguousarray(x_[b]),
            "g": g_,
            "w2": w2,
            "wo": wo_,
        })
    return in_maps


def assemble_out(core_outs):
    """core c, row ib*128 + sb*64 + t -> token sb*NSEQ + ib*512 + c*64 + t."""
    TH = TB // 2
    full = np.empty((B * NSEQ, DIM), np.float32)
    for c in range(NCORES):
        o = core_outs[c]
        for ib in range(IB):
            for sb in range(B):
                dst = sb * NSEQ + ib * 512 + c * TH
                src_r = ib * TB + sb * TH
                full[dst:dst + TH] = o[src_r:src_r + TH]
    return full


_cache = {}


def _get_graph():
    if "nc" not in _cache:
        _cache["nc"] = build_graph()
    return _cache["nc"]


def run_on_hw(in_maps, trace=False, **kw):
    from concourse.bass_utils import run_bass_kernel_spmd
    nc = _get_graph()
    return run_bass_kernel_spmd(
        nc, in_maps, core_ids=list(range(NCORES)), trace=trace, **kw
    )


def kernel(x, g, wq, wkv, wo):
    in_maps = make_in_maps(x, g, wq, wkv, wo)
    res = run_on_hw(in_maps)
    core_outs = [np.asarray(res.results[c]["out"], np.float32)
                 for c in range(NCORES)]
    return assemble_out(core_outs).reshape(B, NSEQ, DIM)


# revision 23
# speedup vs baseline: 1.1872x; 1.0942x over previous
"""Distributed Trainium2 kernel for nn_Attention_29832842838194.

LayerNorm (stats over the *sequence* axis) -> QKV projection -> 8-head
attention (N=2048, d_head=64) -> output projection, on 8 NeuronCores.

Sharding (v3 — one batch + two heads per core, collective-free front):
  - core c owns batch c//4 and heads {2*(c%4), 2*(c%4)+1}. It loads the
    FULL x of its batch (8.4MB), computes LN stats locally (no
    AllReduce!), transposes x on the PE (needed for QKV anyway; the
    stats summation rides on ones-matmuls into PSUM), and projects
    q/k/v for its two heads over all 2048 tokens from a 384-column
    weight slice. Attention then needs NO resharding at all.
  - sims pack the two heads onto PE row halves (head0 rows 0..63,
    head1 rows 64..127) so both stream concurrently; exp on ScalarE
    covers both heads per [128,1024] tile; PV accumulates per head
    with a ones row for the softmax denominator.
  - output: one small AllToAll per 512-query i-block within the 4-core
    batch group (fires as soon as that i-block finishes, fully hidden
    under the next i-block's attention); postprocess (rownorm + out
    projection) trickles into PE slack between sim/PV work.
  - a tiny dummy AllReduce fires at t~0 so the one-time global comm
    rendezvous (~44us) runs concurrent with the whole local prologue.

Numerics: all matmuls bf16 with fp32 PSUM accumulation.

The kernel is self-contained: shapes are hardcoded to the problem spec.
"""

import numpy as np

# -------- problem constants (hardcoded per spec) --------
B = 2
NSEQ = 2048   # sequence length per batch (= tokens per core)
DIM = 1024
HEADS = 8
DHEAD = 64
INNER = HEADS * DHEAD  # 512
EPS = 1e-5
NCORES = 8
P = 128

SCALE = DHEAD ** -0.5  # 0.125
DC = DIM // P          # 8 d-model chunks
XT = NSEQ // P         # 16 x token-subtiles per core
IB = NSEQ // 512       # 4 i-blocks of 512 queries
JC = NSEQ // P         # 16 j-chunks of 128 keys
DE = DHEAD + 1         # head dims + denominator row
W2C = 6 * DHEAD        # 384 projection columns per core (k0 k1 q0 q1 v0 v1)
GS = NCORES // B       # 4 cores per batch group
TB = 512 // GS         # 128 tokens per core per i-block after out-A2A


def build_body(tc, outs, ins, dbg=False):
    """Emit the per-core program. outs/ins are dicts of DRAM APs."""
    import concourse.mybir as mybir
    from concourse.masks import make_identity

    dt = mybir.dt
    AF = mybir.ActivationFunctionType
    ALU = mybir.AluOpType
    nc = tc.nc

    NTOK = float(NSEQ)
    RG = [list(range(NCORES))]
    RGB = [list(range(GS)), list(range(GS, NCORES))]

    x = ins["x"]      # [NSEQ, DIM] f32 (this core's full batch)
    g = ins["g"]      # [DIM] f32
    w2 = ins["w2"]    # [DIM, 384] f32: columns [k_h0 k_h1 q_h0 q_h1 v_h0 v_h1]
    wo = ins["wo"]    # [INNER, DIM] f32 (replicated)
    out = outs["out"]  # [IB*TB, DIM] f32 (rows: ib*TB + t)

    with (
        tc.tile_pool(name="persist", bufs=1) as pp,
        tc.tile_pool(name="work", bufs=3) as pool,
        tc.tile_pool(name="xq", bufs=6) as xq,
        tc.tile_pool(name="xb", bufs=8) as xb,
        tc.tile_pool(name="work2", bufs=2) as pool2,
        tc.tile_pool(name="pref", bufs=6) as pref,
        tc.tile_pool(name="psA", bufs=2, space="PSUM") as psA,
        tc.tile_pool(name="psB", bufs=2, space="PSUM") as psB,
        tc.tile_pool(name="psC", bufs=1, space="PSUM") as psC,
        tc.tile_pool(name="dram", bufs=1, space="DRAM") as dram,
    ):
        # -------- dummy rendezvous collective (absorbs launch skew /
        # one-time comm setup while the local prologue runs)
        dummy_in = dram.tile([1, 8], dt.float32, tag="dmyi")
        dummy_out = dram.tile([1, 8], dt.float32, tag="dmyo")
        nc.gpsimd.collective_compute(
            "AllReduce", ALU.add, replica_groups=RG,
            ins=[dummy_in.opt()], outs=[dummy_out.opt()],
        )

        # -------- constants
        ident = pp.tile([P, P], dt.bfloat16)
        make_identity(nc, ident)
        g_sb = pp.tile([P, DC], dt.float32)
        nc.sync.dma_start(g_sb[:], g.rearrange("(c p) -> p c", p=P))
        ones1 = pp.tile([P, 1], dt.bfloat16)
        nc.vector.memset(ones1[:], 1.0)
        # head-broadcast selector for the rownorm: sel[h, kc, m] = 1 iff
        # inner element kc*128+m belongs to head h (= 2*kc + (m >= 64))
        sel_np = np.zeros((HEADS, INNER // P, P), np.float32)
        for kc in range(INNER // P):
            sel_np[2 * kc, kc, 0:DHEAD] = 1.0
            sel_np[2 * kc + 1, kc, DHEAD:P] = 1.0
        sel_dram = nc.inline_tensor(sel_np, name="selmat")
        sel = pp.tile([HEADS, INNER // P, P], dt.float32)
        nc.sync.dma_start(sel[:], sel_dram.ap())

        # -------- weights: w2 slice early on the scalar queue (needed at
        # QKV start); wo is loaded after the x stream (needed only by the
        # out-projection, ~40us later)
        w2_bf = pp.tile([P, DC, W2C], dt.bfloat16)
        for kc in range(DC):
            wl = pool.tile([P, W2C], dt.float32, tag="wload")
            nc.scalar.dma_start(wl[:], w2[kc * P:(kc + 1) * P, :])
            nc.vector.tensor_copy(w2_bf[:, kc, :], wl[:])
        wo_bf = pp.tile([P, INNER // P, DIM], dt.bfloat16)

        # -------- phase 0: x load -> cast -> transpose -> xT; LN stats
        # accumulate in PSUM via ones-matmuls on the natural layout.
        # xT[p, t, dc, j] = x[t*128+j, dc*128+p] — per-tile-contiguous so
        # the PSUM drain is a fast 2D copy; QKV reads it with a 3D AP.
        xT = pp.tile([P, XT, DC, P], dt.bfloat16)
        # stats accumulators: PSUM matmul outputs must sit at partition
        # 0/32/64, so (sum, sumsq) for each 512-col half live at rows 0
        # and 64 of two [P, 512] tiles.
        st_ps = [psC.tile([P, 512], dt.float32, tag=f"acc{h}",
                          name=f"st_ps{h}") for h in range(2)]
        for t in range(XT):
            x_f = xq.tile([P, DIM], dt.float32, tag="xload")
            nc.sync.dma_start(x_f[:], x[t * P:(t + 1) * P, :])
            x_bf = xb.tile([P, DIM], dt.bfloat16, tag="xbf")
            if t % 2 == 0:
                nc.vector.tensor_copy(x_bf[:], x_f[:])
            else:
                nc.scalar.copy(x_bf[:], x_f[:])
            # squares on the DVE (bf16 2x mode beats ACT here)
            sq = xb.tile([P, DIM], dt.bfloat16, tag="sq")
            nc.vector.tensor_tensor(sq[:], x_bf[:], x_bf[:], ALU.mult)
            for half in range(2):
                nc.tensor.matmul(
                    st_ps[half][0:1, :], ones1[:],
                    x_bf[:, half * 512:(half + 1) * 512],
                    start=(t == 0), stop=(t == XT - 1),
                )
                nc.tensor.matmul(
                    st_ps[half][DHEAD:DHEAD + 1, :], ones1[:],
                    sq[:, half * 512:(half + 1) * 512],
                    start=(t == 0), stop=(t == XT - 1),
                )
            tp = psB.tile([P, DIM], dt.bfloat16, tag="tr")
            for dc in range(DC):
                nc.tensor.transpose(
                    tp[:, dc * P:(dc + 1) * P],
                    x_bf[:, dc * P:(dc + 1) * P],
                    ident[:],
                )
            xdst = xT[:, t].rearrange("p dc j -> p (dc j)")
            if t % 2 == 0:
                nc.vector.tensor_copy(xdst, tp[:])
            else:
                nc.scalar.copy(xdst, tp[:])
        # wo loads go behind the x stream on the scalar queue
        for kc in range(INNER // P):
            wol = pool.tile([P, DIM], dt.float32, tag="woload")
            nc.scalar.dma_start(wol[:], wo[kc * P:(kc + 1) * P, :])
            nc.gpsimd.tensor_copy(wo_bf[:, kc, :], wol[:])

        # -------- LN coefficients (local, no collective!)
        # free->partition reshape must bounce through DRAM (SBUF APs
        # can't step partitions through free memory; PSUM can't feed
        # DMA directly, so hop PSUM -> SBUF -> DRAM -> stats).
        st_sb = [pool.tile([P, 512], dt.float32, tag="stc",
                           name=f"st_sb{h}") for h in range(2)]
        for h in range(2):
            nc.vector.tensor_copy(st_sb[h][:], st_ps[h][:])
        st_dram = dram.tile([4, 512], dt.float32, tag="stdr")
        for h in range(2):
            nc.sync.dma_start(st_dram[h], st_sb[h][0:1, :])
            nc.sync.dma_start(st_dram[2 + h],
                              st_sb[h][DHEAD:DHEAD + 1, :])
        stats = pp.tile([P, 2 * DC], dt.float32)
        # stats[p, dc] = sum[dc*128+p]; stats[p, DC+dc] = sumsq[dc*128+p]
        nc.sync.dma_start(
            stats[:, 0:DC],
            st_dram[0:2].rearrange("h (q p) -> p (h q)", p=P),
        )
        nc.sync.dma_start(
            stats[:, DC:2 * DC],
            st_dram[2:4].rearrange("h (q p) -> p (h q)", p=P),
        )
        mean = pp.tile([P, DC], dt.float32)
        nc.vector.tensor_scalar_mul(mean[:], stats[:, 0:DC], 1.0 / NTOK)
        e2 = pp.tile([P, DC], dt.float32)
        nc.vector.tensor_scalar_mul(e2[:], stats[:, DC:2 * DC], 1.0 / NTOK)
        msq = pp.tile([P, DC], dt.float32)
        nc.vector.tensor_tensor(msq[:], mean[:], mean[:], ALU.mult)
        vareps = pp.tile([P, DC], dt.float32)
        nc.vector.tensor_tensor(vareps[:], e2[:], msq[:], ALU.subtract)
        nc.vector.tensor_scalar_add(vareps[:], vareps[:], EPS)
        rvar = pp.tile([P, DC], dt.float32)
        nc.vector.reciprocal(rvar[:], vareps[:])
        rstd = pp.tile([P, DC], dt.float32)
        nc.scalar.activation(rstd[:], rvar[:], AF.Sqrt)
        A2 = pp.tile([P, DC], dt.float32)
        nc.vector.tensor_tensor(A2[:], rstd[:], g_sb[:], ALU.mult)
        C2 = pp.tile([P, DC], dt.float32)
        nc.vector.tensor_tensor(C2[:], mean[:], A2[:], ALU.mult)
        nc.vector.tensor_scalar_mul(C2[:], C2[:], -1.0)
        # preload the exp table while the PE chews on QKV
        junk = pp.tile([1, DC], dt.float32)
        nc.scalar.activation(junk[:], A2[0:1, :], AF.Exp)
        if dbg:
            nc.sync.dma_start(outs["dbg_stats"], stats[:])
            nc.sync.dma_start(outs["dbg_A2"], A2[:])
            nc.sync.dma_start(outs["dbg_C2"], C2[:])

        # fold LN into the projection: q = x @ (A*w2) + (-mean) @ (A*w2),
        # so xT stays RAW and the per-token normalize pass disappears.
        for kc in range(DC):
            nc.vector.tensor_scalar(
                w2_bf[:, kc, :], w2_bf[:, kc, :],
                A2[:, kc:kc + 1], None, ALU.mult,
            )
        negmu = pp.tile([P, DC], dt.bfloat16)
        nc.vector.tensor_scalar(negmu[:], mean[:], -1.0, None, ALU.mult)
        bp = psB.tile([1, W2C], dt.float32, tag="tr")
        for kc in range(DC):
            nc.tensor.matmul(
                bp[:], negmu[:, kc:kc + 1], w2_bf[:, kc, :],
                start=(kc == 0), stop=(kc == DC - 1),
            )
        bp_sb = pool.tile([1, W2C], dt.float32, tag="stc")
        nc.vector.tensor_copy(bp_sb[:], bp[:])
        bias_dram = dram.tile([1, W2C], dt.float32, tag="biasd")
        nc.sync.dma_start(bias_dram[:], bp_sb[:])
        bias_sb = pp.tile([P, 3], dt.float32)
        nc.sync.dma_start(
            bias_sb[:],
            bias_dram[:].rearrange("o (c p) -> p (o c)", p=P),
        )

        # -------- QKV projection for this core's 2 heads over all tokens.
        # w2 columns: [k(128) | q(128) | v(128)], head-major inside each.
        # k/q land transposed ([dims, tokens]) which is exactly the sim
        # layout; v lands as vT and is flipped by 32 tiny DMA transposes.
        kTh = pp.tile([P, NSEQ], dt.bfloat16)
        qTh = pp.tile([P, NSEQ], dt.bfloat16)
        vT = pp.tile([P, NSEQ], dt.bfloat16)
        dsts = [kTh, qTh, vT]
        for blk in range(3):
            for tp2 in range(2):
                ps = psA.tile([P, 1024], dt.float32, tag="sim")
                for half in range(2):
                    tcn = tp2 * 2 + half
                    for kc in range(DC):
                        nc.tensor.matmul(
                            ps[:, half * 512:(half + 1) * 512],
                            w2_bf[:, kc, blk * P:(blk + 1) * P],
                            xT[:, tcn * 4:(tcn + 1) * 4, kc, :],
                            start=(kc == 0), stop=(kc == DC - 1),
                        )
                nc.vector.tensor_scalar(
                    dsts[blk][:, tp2 * 1024:(tp2 + 1) * 1024], ps[:],
                    bias_sb[:, blk:blk + 1], None, ALU.add,
                )

        # vext[h][j, jc, d|1]: per-head value tiles with the ones column
        vext = [pp.tile([P, JC, DE], dt.bfloat16, name=f"vext{h}")
                for h in range(2)]
        for h in range(2):
            nc.gpsimd.memset(vext[h][:, :, DHEAD:DE], 1.0)
        for tv in range(JC):
            vtp = psB.tile([P, P], dt.bfloat16, tag="tr")
            nc.tensor.transpose(vtp[:], vT[:, tv * P:(tv + 1) * P],
                                ident[:])
            for h in range(2):
                nc.vector.tensor_copy(vext[h][:, tv, 0:DHEAD],
                                      vtp[:, h * DHEAD:(h + 1) * DHEAD])
        if dbg:
            nc.sync.dma_start(outs["dbg_xn"], xT[:])
            nc.sync.dma_start(outs["dbg_kT"], kTh[:])
            nc.sync.dma_start(outs["dbg_qT"], qTh[:])
            nc.sync.dma_start(outs["dbg_v0"], vext[0][:])
            nc.sync.dma_start(outs["dbg_v1"], vext[1][:])

        # -------- attention + per-i-block out A2A, postprocess trickled
        def attn_block(ib, todo=()):
            todo = list(todo)
            i0 = ib * 512
            ot = [psC.tile([DE, 512], dt.float32, tag=f"acc{h}",
                          name=f"ot{h}") for h in range(2)]
            for jc in range(JC):
                if jc % 4 == 3 and todo:
                    todo.pop(0)()
                sp = psA.tile([P, 1024], dt.float32, tag="sim")
                for h in range(2):
                    rsl = slice(h * DHEAD, (h + 1) * DHEAD)
                    nc.tensor.matmul(
                        sp[:, h * 512:(h + 1) * 512],
                        kTh[rsl, jc * P:(jc + 1) * P],
                        qTh[rsl, i0:i0 + 512],
                        start=True, stop=True,
                    )
                et = pref.tile([P, 1024], dt.bfloat16, tag="exp")
                nc.scalar.activation(et[:], sp[:], AF.Exp, scale=SCALE)
                for h in range(2):
                    nc.tensor.matmul(
                        ot[h][:], vext[h][:, jc, :],
                        et[:, h * 512:(h + 1) * 512],
                        start=(jc == 0), stop=(jc == JC - 1),
                    )
            ao = pool2.tile([DE, 2, 512], dt.bfloat16, tag="ao")
            for h in range(2):
                nc.vector.tensor_copy(ao[:, h, :], ot[h][:])
            return ao

        def out_a2a(ib, ao):
            # all-8 AllToAll: destination core d gets tokens
            # [ib*512 + d*64, +64) of BOTH batches (rows 0..3 = batch-0
            # heads, rows 4..7 = batch-1 heads) -- mesh needs >4 cores.
            TH = TB // 2
            a2a_in = dram.tile([NCORES, 2, DE, TH], dt.bfloat16,
                               tag=f"oa{ib}")
            for h in range(2):
                nc.sync.dma_start(
                    a2a_in[:, h].rearrange("r d t -> d r t"),
                    ao[:, h, :].rearrange("d (r t) -> d r t", r=NCORES),
                )
            a2a_out = dram.tile([NCORES, 2, DE, TH], dt.bfloat16,
                                tag=f"ob{ib}")
            nc.gpsimd.collective_compute(
                "AllToAll", ALU.bypass, replica_groups=RG,
                ins=[a2a_in.opt()], outs=[a2a_out.opt()],
            )
            return a2a_out

        def pp_stages(ib, a2a_out):
            """Postprocess one i-block's received tokens (64 per batch,
            packed side by side into 128 columns), split into stages so
            the PE work trickles into attention slack."""
            st = {}
            TH = TB // 2

            def s_gather():
                # ao_g[hh*64+d, s, sb*64+t] = a2a_out[sb*4+s, hh, d, t]
                ao_g = pool2.tile([P, GS, TB], dt.bfloat16, tag="aog")
                for hh in range(2):
                    for sb in range(2):
                        nc.sync.dma_start(
                            ao_g[hh * DHEAD:(hh + 1) * DHEAD, :,
                                 sb * TH:(sb + 1) * TH],
                            a2a_out[sb * GS:(sb + 1) * GS, hh, 0:DHEAD,
                                    :].rearrange("s d t -> d s t"),
                        )
                rn = pool2.tile([HEADS, TB], dt.bfloat16, tag="rn")
                for sb in range(2):
                    nc.sync.dma_start(
                        rn[:, sb * TH:(sb + 1) * TH],
                        a2a_out[sb * GS:(sb + 1) * GS, :, DHEAD,
                                :].rearrange("s h t -> (s h) t"),
                    )
                rc = pool2.tile([HEADS, TB], dt.float32, tag="rc")
                nc.vector.reciprocal(rc[:], rn[:])
                st["ao_g"], st["rc"] = ao_g, rc

            def s_norm():
                ao_g, rc = st["ao_g"], st["rc"]
                for kc in range(INNER // P):
                    bcp = psB.tile([P, TB], dt.float32, tag="tr")
                    nc.tensor.matmul(bcp[:], sel[:, kc, :], rc[:],
                                     start=True, stop=True)
                    nc.vector.tensor_tensor(
                        ao_g[:, kc, :], ao_g[:, kc, :], bcp[:], ALU.mult
                    )

            def s_proj(nh2):
                ao_g = st["ao_g"]
                op = psB.tile([P, 512], dt.float32, tag="tr")
                for kc in range(INNER // P):
                    nc.tensor.matmul(
                        op[:], ao_g[:, kc, :],
                        wo_bf[:, kc, nh2 * 512:(nh2 + 1) * 512],
                        start=(kc == 0), stop=(kc == INNER // P - 1),
                    )
                out_sb = st.setdefault(
                    "osb", pool2.tile([P, DIM], dt.float32, tag="osb",
                                      name="out_sb"))
                nc.vector.tensor_copy(
                    out_sb[:, nh2 * 512:(nh2 + 1) * 512], op[:])
                if nh2 == DIM // 512 - 1:
                    # rows 0..63 = batch-0 tokens, 64..127 = batch-1
                    nc.sync.dma_start(out[ib * TB:(ib + 1) * TB, :],
                                      out_sb[:])

            return [s_gather, s_norm] + \
                [lambda nh2=nh2: s_proj(nh2) for nh2 in range(DIM // 512)]

        # pp(ib) runs interleaved inside attn(ib+2)'s jc stream: by then
        # its A2A has certainly landed, so the PE queue never blocks on
        # unready collective data (engine queues execute in-order).
        stages = {}
        for ib in range(IB):
            todo = stages.pop(ib - 2, [])
            ao = attn_block(ib, todo)
            o = out_a2a(ib, ao)
            stages[ib] = pp_stages(ib, o)
        for ib in sorted(stages):
            for s in stages[ib]:
                s()


def build_graph(dbg=False):
    import concourse.mybir as mybir
    import concourse.tile as tile
    from concourse import bacc

    dt = mybir.dt
    nc = bacc.Bacc("TRN2", target_bir_lowering=False, debug=False,
                   num_devices=NCORES)
    ins = {
        "x": nc.dram_tensor("x", [NSEQ, DIM], dt.float32,
                            kind="ExternalInput").ap(),
        "g": nc.dram_tensor("g", [DIM], dt.float32,
                            kind="ExternalInput").ap(),
        "w2": nc.dram_tensor("w2", [DIM, W2C], dt.float32,
                             kind="ExternalInput").ap(),
        "wo": nc.dram_tensor("wo", [INNER, DIM], dt.float32,
                             kind="ExternalInput").ap(),
    }
    outs = {
        "out": nc.dram_tensor("out", [IB * TB, DIM], dt.float32,
                              kind="ExternalOutput").ap(),
    }
    if dbg:
        for name, shape, dt_ in (
            ("dbg_stats", [P, 2 * DC], dt.float32),
            ("dbg_A2", [P, DC], dt.float32),
            ("dbg_C2", [P, DC], dt.float32),
            ("dbg_xn", [P, XT, DC, P], dt.bfloat16),
            ("dbg_kT", [P, NSEQ], dt.bfloat16),
            ("dbg_qT", [P, NSEQ], dt.bfloat16),
            ("dbg_v0", [P, JC, DE], dt.bfloat16),
            ("dbg_v1", [P, JC, DE], dt.bfloat16),
        ):
            outs[name] = nc.dram_tensor(name, shape, dt_,
                                        kind="ExternalOutput").ap()
    with tile.TileContext(nc) as tc:
        build_body(tc, outs, ins, dbg=dbg)
    nc.compile()
    return nc


def make_in_maps(x, g, wq, wkv, wo):
    """Shard full inputs into per-core input maps."""
    x_ = np.asarray(x, np.float32)
    g_ = np.ascontiguousarray(np.asarray(g, np.float32))
    wq_ = np.asarray(wq, np.float32)
    wkv_ = np.asarray(wkv, np.float32)
    wo_ = np.ascontiguousarray(np.asarray(wo, np.float32))
    wk_ = wkv_[:, :INNER]
    wv_ = wkv_[:, INNER:]
    in_maps = []
    for c in range(NCORES):
        b, r = divmod(c, GS)
        h0 = 2 * r * DHEAD
        h2 = h0 + 2 * DHEAD
        w2 = np.ascontiguousarray(np.concatenate(
            [wk_[:, h0:h2], wq_[:, h0:h2], wv_[:, h0:h2]], axis=1))
        in_maps.append({
            "x": np.ascontiguousarray(x_[b]),
            "g": g_,
            "w2": w2,
            "wo": wo_,
        })
    return in_maps


def assemble_out(core_outs):
    """core c, row ib*128 + sb*64 + t -> token sb*NSEQ + ib*512 + c*64 + t."""
    TH = TB // 2
    full = np.empty((B * NSEQ, DIM), np.float32)
    for c in range(NCORES):
        o = core_outs[c]
        for ib in range(IB):
            for sb in range(B):
                dst = sb * NSEQ + ib * 512 + c * TH
                src_r = ib * TB + sb * TH
                full[dst:dst + TH] = o[src_r:src_r + TH]
    return full


_cache = {}


def _get_graph():
    if "nc" not in _cache:
        _cache["nc"] = build_graph()
    return _cache["nc"]


def run_on_hw(in_maps, trace=False, **kw):
    from concourse.bass_utils import run_bass_kernel_spmd
    nc = _get_graph()
    return run_bass_kernel_spmd(
        nc, in_maps, core_ids=list(range(NCORES)), trace=trace, **kw
    )


def kernel(x, g, wq, wkv, wo):
    in_maps = make_in_maps(x, g, wq, wkv, wo)
    res = run_on_hw(in_maps)
    core_outs = [np.asarray(res.results[c]["out"], np.float32)
                 for c in range(NCORES)]
    return assemble_out(core_outs).reshape(B, NSEQ, DIM)
